# revision 1
# baseline (speedup 1.0000x reference)
"""Radix-2 DIF variant: contraction halved by pre-combining lag-product halves.

X[k, 2t]   = sum_{m<512} (R[k,m]+R[k,m+512]) * w512^{mt}
X[k, 2t+1] = sum_{m<512} (R[k,m]-R[k,m+512]) * w^m * w512^{mt}

Rsum/Rdiff are built on VectorE from sliding-window tiles (negative free-step
reads are legal on DVE), so the matmul stationaries are plain ascending slices
and the output comes out with k ascending (no J-flip on the direct path).
The w^m twiddle and the f-axis fftshift are baked into constant DFT tables
(stationary-free moving operands, resident in SBUF).
"""

import numpy as np

import bass_rust
import concourse.bass as bass
import concourse.mybir as mybir
import concourse.tile as tile
import concourse.bass_utils as bass_utils

B, N = 16, 1024
NCORES = 8
BPC = B // NCORES
NKB = 5  # k-blocks: k in [0, 640)
DS_LEN = 2176

f32 = mybir.dt.float32
f32r = mybir.dt.float32r
ALU = mybir.AluOpType


def _split_excess_waits(nc):
    for f in nc.m.functions:
        for blk in f.blocks:
            insts = list(blk.instructions)
            new_insts = []
            changed = False
            for inst in insts:
                si = inst.sync_info
                waits = list(si.on_wait) if (si is not None and si.on_wait) else []
                keep_n = 0 if isinstance(inst, mybir.InstDrain) else 1
                if len(waits) > keep_n:
                    changed = True
                    extra = waits[: len(waits) - keep_n]
                    keep = waits[len(waits) - keep_n:]
                    for w in extra:
                        nop = mybir.InstNoOp(
                            name=nc.get_next_instruction_name(), ins=[], outs=[]
                        )
                        nop.engine = inst.engine
                        nop.sync_info = bass_rust.SyncInfo(on_wait=[w], on_update=[])
                        new_insts.append(nop)
                    inst.sync_info = bass_rust.SyncInfo(
                        on_wait=keep,
                        on_update=list(si.on_update) if si.on_update else [],
                    )
                new_insts.append(inst)
            if changed:
                blk.instructions = new_insts
    return nc


TABNAMES = ["tec", "tes", "tesn", "toc", "tos", "tosn"]


def build_nc():
    nc = bass.Bass("TRN2", target_bir_lowering=False, debug=False)

    dsr = nc.dram_tensor("dsr", [BPC, DS_LEN], f32r, kind="ExternalInput")
    dsi = nc.dram_tensor("dsi", [BPC, DS_LEN], f32r, kind="ExternalInput")
    dsni = nc.dram_tensor("dsni", [BPC, DS_LEN], f32r, kind="ExternalInput")
    scols = nc.dram_tensor("scols", [BPC, 128, 16], f32, kind="ExternalInput")
    tabs = {
        nm: nc.dram_tensor(nm, [512, 512], f32r, kind="ExternalInput")
        for nm in TABNAMES
    }
    jmat = nc.dram_tensor("jmat", [128, 128], f32r, kind="ExternalInput")
    out = nc.dram_tensor("out", [BPC, N, N], f32, kind="ExternalOutput")

    with tile.TileContext(nc) as tc:
        with (
            tc.tile_pool(name="const", bufs=1) as constp,
            tc.tile_pool(name="tp", bufs=1) as tp,
            tc.tile_pool(name="rp", bufs=1) as rp,
            tc.tile_pool(name="tmp", bufs=2) as tmpp,
            tc.tile_pool(name="u", bufs=1) as up,
            tc.tile_pool(name="chi", bufs=1) as chip,
            tc.tile_pool(name="st", bufs=2) as stp,
            tc.tile_pool(name="ms", bufs=1) as msp,
            tc.tile_pool(name="mj", bufs=2) as mjp,
            tc.tile_pool(name="sm", bufs=1) as smp,
            tc.tile_pool(name="ps", bufs=2, space="PSUM") as psp,
        ):
            tJ = constp.tile([128, 128], f32r, tag="jmat")
            nc.scalar.dma_start(tJ[:], jmat[:])
            # resident DFT tables, per 128-chunk of m
            TT = {}
            k = 0
            for q in range(4):  # q-major: chunk-0 tables land first
                for nm in TABNAMES:
                    t = constp.tile([128, 512], f32r, tag=f"{nm}{q}")
                    TT[(nm, q)] = t
            def load_tab(nm, q, eng):
                eng.dma_start(TT[(nm, q)][:], tabs[nm][q * 128:(q + 1) * 128, :])

            def emit_load(b):
                s = {"b": b, "chis": [], "R": [None] * 4}
                scol = smp.tile([128, 16], f32, tag=f"scol{b}")
                nc.sync.dma_start(scol[:], scols[b])
                s["scol"] = scol
                Tsr = tp.tile([128, 1536], f32r, tag="tsr")
                Tsi = tp.tile([128, 1536], f32r, tag="tsi")
                Tnsi = tp.tile([128, 1536], f32r, tag="tnsi")
                nc.sync.dma_start(Tsr[:], bass.AP(dsr, b * DS_LEN + 385, [[1, 128], [1, 1536]]))
                nc.scalar.dma_start(Tsi[:], bass.AP(dsi, b * DS_LEN + 385, [[1, 128], [1, 1536]]))
                nc.gpsimd.dma_start(Tnsi[:], bass.AP(dsni, b * DS_LEN + 385, [[1, 128], [1, 1536]]))
                s["T"] = (Tsr, Tsi, Tnsi)
                rowall = smp.tile([1, 640], f32, tag=f"rowall{b}")
                s["rowall"] = rowall
                return s

            def win(T, off):
                # [p, kk] -> T[p, off - kk], kk in [0, 640)
                ap = T[:]
                return bass.AP(ap.tensor, ap.offset + off, [ap.ap[0], [-1, 640]])

            def emit_rbuild(s, qs, lo=0, hi=640):
                # R^T[m, kk] = s[m] * conj(s)[(m-kk)%N]; sum/diff of m and m+512.
                # Built in k-column slices so early k-blocks unblock sooner.
                Tsr, Tsi, Tnsi = s["T"]
                scol = s["scol"]
                n = hi - lo
                for q in qs:
                    m0 = 128 * q
                    terms = []
                    for half, woff in ((0, 1024 + m0), (1, 1536 + m0)):
                        sr_c = scol[:, q + 4 * half:q + 4 * half + 1]
                        si_c = scol[:, 8 + q + 4 * half:9 + q + 4 * half]
                        def w(T):
                            ap = T[:]
                            return bass.AP(ap.tensor, ap.offset + woff - 385 - lo, [ap.ap[0], [-1, n]])
                        w_sr, w_si, w_nsi = w(Tsr), w(Tsi), w(Tnsi)
                        a = tmpp.tile([128, 640], f32, tag="ta")
                        ur = up.tile([128, 640], f32, tag=f"ur{half}")
                        # Rr = sr_m*csr + si_m*si_win
                        nc.vector.tensor_scalar_mul(a[:, 0:n], w_sr, sr_c)
                        nc.vector.scalar_tensor_tensor(
                            ur[:, 0:n], w_si, si_c, a[:, 0:n], op0=ALU.mult, op1=ALU.add
                        )
                        b2 = tmpp.tile([128, 640], f32, tag="tb")
                        ui = up.tile([128, 640], f32, tag=f"ui{half}")
                        # Ri = si_m*csr - sr_m*si_win  (= si_m*csr + sr_m*(-si_win))
                        nc.vector.tensor_scalar_mul(b2[:, 0:n], w_nsi, sr_c)
                        nc.vector.scalar_tensor_tensor(
                            ui[:, 0:n], w_sr, si_c, b2[:, 0:n], op0=ALU.mult, op1=ALU.add
                        )
                        terms.append((ur, ui))
                    (u1r, u1i), (u2r, u2i) = terms
                    if lo == 0:
                        qt = f"0_{s['b'] % 2}" if q == 0 else str(q)
                        rsr = rp.tile([128, 640], f32r, tag=f"rsr{qt}")
                        rsi = rp.tile([128, 640], f32r, tag=f"rsi{qt}")
                        rdr = rp.tile([128, 640], f32r, tag=f"rdr{qt}")
                        rdi = rp.tile([128, 640], f32r, tag=f"rdi{qt}")
                    else:
                        rsr, rsi, rdr, rdi = s["R"][q]
                    nc.vector.scalar_tensor_tensor(
                        rsr[:, lo:hi], u1r[:, 0:n], 1.0, u2r[:, 0:n], op0=ALU.mult, op1=ALU.add)
                    nc.vector.scalar_tensor_tensor(
                        rdr[:, lo:hi], u1r[:, 0:n], 1.0, u2r[:, 0:n], op0=ALU.mult, op1=ALU.subtract)
                    nc.vector.scalar_tensor_tensor(
                        rsi[:, lo:hi], u1i[:, 0:n], 1.0, u2i[:, 0:n], op0=ALU.mult, op1=ALU.add)
                    nc.vector.scalar_tensor_tensor(
                        rdi[:, lo:hi], u1i[:, 0:n], 1.0, u2i[:, 0:n], op0=ALU.mult, op1=ALU.subtract)
                    s["R"][q] = (rsr, rsi, rdr, rdi)

            def emit_kblock(b, s, kb):
                c = 128 * kb
                xre = psp.tile([128, 512], f32, tag="xre")
                xie = psp.tile([128, 512], f32, tag="xie")
                xro = psp.tile([128, 512], f32, tag="xro")
                xio = psp.tile([128, 512], f32, tag="xio")
                for q in range(4):
                    rsr, rsi, rdr, rdi = s["R"][q]
                    first = q == 0
                    last = q == 3
                    psr = rsr[:, c:c + 128]
                    psi = rsi[:, c:c + 128]
                    pdr = rdr[:, c:c + 128]
                    pdi = rdi[:, c:c + 128]
                    nc.tensor.matmul(xre[:], psr, TT[("tec", q)][:], start=first, stop=False)
                    nc.tensor.matmul(xie[:], psi, TT[("tec", q)][:], start=first, stop=False)
                    nc.tensor.matmul(xro[:], pdr, TT[("toc", q)][:], start=first, stop=False)
                    nc.tensor.matmul(xio[:], pdi, TT[("toc", q)][:], start=first, stop=False)
                    nc.tensor.matmul(xre[:], psi, TT[("tes", q)][:], start=False, stop=last)
                    nc.tensor.matmul(xie[:], psr, TT[("tesn", q)][:], start=False, stop=last)
                    nc.tensor.matmul(xro[:], pdi, TT[("tos", q)][:], start=False, stop=last)
                    nc.tensor.matmul(xio[:], pdr, TT[("tosn", q)][:], start=False, stop=last)

                chi_t = chip.tile([128, N], f32, tag=f"chi{(5 * b + kb) % 6}")
                tmax2 = smp.tile([128, 2], f32, tag=f"tmax{b}")
                for parity, (xr, xi) in ((0, (xre, xie)), (1, (xro, xio))):
                    sqa = tmpp.tile([128, 512], f32, tag="ta")
                    sqb = tmpp.tile([128, 512], f32, tag="tb")
                    nc.scalar.square(sqa[:], xr[:])
                    nc.scalar.square(sqb[:], xi[:])
                    cap = chi_t[:]
                    strided = bass.AP(cap.tensor, cap.offset + parity, [cap.ap[0], [2, 512]])
                    nc.vector.tensor_add(strided, sqa[:], sqb[:])
                    nc.vector.tensor_reduce(
                        tmax2[:, parity:parity + 1], strided,
                        axis=mybir.AxisListType.X, op=ALU.max,
                    )
                tmax1 = smp.tile([128, 1], f32, tag=f"tmax1_{b}")
                nc.vector.tensor_max(tmax1[:], tmax2[:, 0:1], tmax2[:, 1:2])
                # transpose this block's per-partition max into the row
                # accumulator now, so the final reduce is one short chain
                nc.sync.dma_start(s["rowall"][0:1, 128 * kb:128 * (kb + 1)], tmax1[:])
                s["chis"].append(chi_t)

            def emit_finalize(b, s):
                gmax = smp.tile([1, 1], f32, tag=f"gmax{b}")
                nc.vector.tensor_reduce(
                    gmax[:], s["rowall"][:], axis=mybir.AxisListType.X, op=ALU.max
                )
                bmax = smp.tile([128, 1], f32, tag=f"bmax{b}")
                nc.sync.dma_start(
                    bmax[:], bass.AP(gmax[:].tensor, gmax[:].offset, [[1, 1], [0, 128]])
                )
                binv = smp.tile([128, 1], f32, tag=f"binv{b}")
                nc.vector.reciprocal(binv[:], bmax[:])
                s["binv"] = binv

            def emit_direct(b, s, kbs):
                # k is already ascending: scale + store
                binv = s["binv"]
                for kb in kbs:
                    stg = stp.tile([128, N], f32, tag="stg")
                    nc.vector.tensor_scalar_mul(stg[:], s["chis"][kb][:], binv[:])
                    r0 = (128 * kb + 512) % N
                    eng = nc.sync if kb % 2 == 0 else nc.scalar
                    eng.dma_start(out[b, r0:r0 + 128, :], stg[:])

            def emit_mirror_flip(b, s, kbs):
                # f-reverse chi[k2] rows (k2 in [1,384] live in kb 0..3)
                s.setdefault("ms", {})
                for kb in kbs:
                    chi_t = s["chis"][kb]
                    ms = msp.tile([128, N], f32r, tag=f"ms{kb % 2}")
                    ap = chi_t[:]
                    rev = bass.AP(ap.tensor, ap.offset + 1023, [ap.ap[0], [-1, 1023]])
                    nc.vector.tensor_copy(ms[:, 0:1], chi_t[:, 0:1])
                    nc.vector.tensor_copy(ms[:, 1:1024], rev)
                    s["ms"][kb] = ms

            def emit_mirror_jcopy(b, s, kbs):
                # J-flip (k asc -> desc) + unscaled PSUM->SBUF copy; no binv
                # dependency, so this overlaps the remaining k-blocks
                s.setdefault("mj", {})
                for kb in kbs:
                    ms = s["ms"][kb]
                    mj = mjp.tile([128, N], f32, tag=f"mj{kb % 2}")
                    for h in range(2):
                        hs = 512 * h
                        jy = psp.tile([128, 512], f32, tag=("xre" if h == 0 else "xro"))
                        nc.tensor.matmul(jy[:], tJ[:], ms[:, hs:hs + 512], start=True, stop=True)
                        nc.scalar.copy(mj[:, hs:hs + 512], jy[:])
                    s["mj"][kb] = mj

            def emit_mirror_store(b, s, kbs):
                # scale in place once 1/max is known, then store:
                # source partition r holds k2 = c+127-r -> dest row 385-c+r
                binv = s["binv"]
                for kb in kbs:
                    c = 128 * kb
                    mj = s["mj"][kb]
                    nc.scalar.mul(mj[:], mj[:], binv[:])
                    eng = nc.scalar if kb % 2 == 0 else nc.sync
                    if kb == 0:
                        eng.dma_start(out[b, 385:512, :], mj[0:127, :])
                    elif kb == 3:
                        eng.dma_start(out[b, 128:129, :], mj[127:128, :])
                    else:
                        r0 = 385 - c
                        eng.dma_start(out[b, r0:r0 + 128, :], mj[:])

            # --- pipelined schedule
            s0 = emit_load(0)
            for nm in TABNAMES:
                load_tab(nm, 0, nc.sync if nm in ("tec", "tes", "tesn") else nc.scalar)
            emit_rbuild(s0, [0])
            for q in (1, 2, 3):
                for i, nm in enumerate(TABNAMES):
                    load_tab(nm, q, (nc.sync, nc.scalar, nc.gpsimd)[i % 3])
            emit_rbuild(s0, [1, 2, 3], 0, 320)
            emit_rbuild(s0, [1, 2, 3], 320, 640)
            for kb in range(4):
                emit_kblock(0, s0, kb)
            s1 = emit_load(1)
            emit_rbuild(s1, [0])
            emit_kblock(0, s0, 4)
            emit_finalize(0, s0)
            emit_rbuild(s1, [1, 2, 3], 0, 320)
            emit_rbuild(s1, [1, 2, 3], 320, 640)
            emit_mirror_flip(0, s0, [0, 1])
            emit_mirror_jcopy(0, s0, [0, 1])
            emit_kblock(1, s1, 0)
            emit_kblock(1, s1, 1)
            emit_direct(0, s0, [0, 1])
            emit_mirror_store(0, s0, [0, 1])
            emit_kblock(1, s1, 2)
            emit_mirror_flip(0, s0, [2, 3])
            emit_mirror_jcopy(0, s0, [2, 3])
            emit_direct(0, s0, [2, 3])
            emit_mirror_store(0, s0, [2, 3])
            emit_kblock(1, s1, 3)
            emit_direct(0, s0, [4])
            emit_mirror_flip(1, s1, [0, 1])
            emit_mirror_jcopy(1, s1, [0, 1])
            emit_mirror_flip(1, s1, [2, 3])
            emit_mirror_jcopy(1, s1, [2, 3])
            emit_kblock(1, s1, 4)
            emit_finalize(1, s1)
            emit_direct(1, s1, [0, 1, 2, 3, 4])
            emit_mirror_store(1, s1, [0, 1, 2, 3])

    _split_excess_waits(nc)
    return nc


_NC_CACHE = {}


def _get_nc():
    if "nc" not in _NC_CACHE:
        _NC_CACHE["nc"] = build_nc()
    return _NC_CACHE["nc"]


def _get_tables():
    if "tabs" not in _NC_CACHE:
        m = np.arange(512, dtype=np.float64)[:, None]
        tp_ = np.arange(512, dtype=np.float64)[None, :]
        t_of = (tp_ + 256) % 512
        ang_e = 2.0 * np.pi * ((m * t_of) % 512) / 512
        ang_o = ang_e + 2.0 * np.pi * m / 1024
        tabs = {
            "tec": np.cos(ang_e).astype(np.float32),
            "tes": np.sin(ang_e).astype(np.float32),
            "toc": np.cos(ang_o).astype(np.float32),
            "tos": np.sin(ang_o).astype(np.float32),
        }
        tabs["tesn"] = -tabs["tes"]
        tabs["tosn"] = -tabs["tos"]
        _NC_CACHE["tabs"] = (tabs, np.eye(128, dtype=np.float32)[::-1].copy())
    return _NC_CACHE["tabs"]


def kernel(s_real: np.ndarray, s_imag: np.ndarray) -> np.ndarray:
    s_real = np.asarray(s_real, dtype=np.float32)
    s_imag = np.asarray(s_imag, dtype=np.float32)
    tabs, jnp_ = _get_tables()
    nc = _get_nc()

    in_maps = []
    for core in range(NCORES):
        sl = slice(core * BPC, (core + 1) * BPC)
        sr = s_real[sl]
        si = s_imag[sl]
        dsr = np.tile(sr, (1, 3))[:, :DS_LEN].copy()
        dsi_ = np.tile(si, (1, 3))[:, :DS_LEN].copy()
        scols = np.concatenate(
            [
                sr.reshape(BPC, 8, 128).transpose(0, 2, 1),
                si.reshape(BPC, 8, 128).transpose(0, 2, 1),
            ],
            axis=2,
        ).astype(np.float32).copy()
        im = {"dsr": dsr, "dsi": dsi_, "dsni": -dsi_, "scols": scols, "jmat": jnp_}
        im.update(tabs)
        in_maps.append(im)

    res = bass_utils.run_bass_kernel_spmd(nc, in_maps, core_ids=list(range(NCORES)))
    return np.concatenate([r["out"] for r in res.results], axis=0)



# revision 2
# speedup vs baseline: 1.1777x; 1.1777x over previous
"""Radix-4 DIF ambiguity kernel.

Per batch: u_c = s[m]*conj(s[m-k]) sliding-window products (DVE, bf16),
FFT4 combine over c (DVE, bf16), then 4 branch DFT-256 matmuls with
re/im-concatenated bf16 tables (PE, 512-wide moving), |X|^2 via ACT squares
+ DVE/Pool pair adds. Normalization is exact-by-construction (Cauchy-Schwarz:
max chi = (sum|s|^2)^2) and folded into a host prescale of s. Only k in
[0,512) is computed on device; row k=512 and the mirror half-plane
chi[k,f] = chi[N-k, -f] are assembled during host-side unsharding.
"""

import numpy as np
import ml_dtypes

import bass_rust
import concourse.bass as bass
import concourse.mybir as mybir
import concourse.tile as tile
import concourse.bass_utils as bass_utils

B, N = 16, 1024
NCORES = 8
BPC = B // NCORES
K = 512
DS_LEN = 2176

f32 = mybir.dt.float32
bf16 = mybir.dt.bfloat16
ALU = mybir.AluOpType


def _split_excess_waits(nc):
    for f in nc.m.functions:
        for blk in f.blocks:
            insts = list(blk.instructions)
            new_insts = []
            changed = False
            for inst in insts:
                si = inst.sync_info
                waits = list(si.on_wait) if (si is not None and si.on_wait) else []
                keep_n = 0 if isinstance(inst, mybir.InstDrain) else 1
                if len(waits) > keep_n:
                    changed = True
                    extra = waits[: len(waits) - keep_n]
                    keep = waits[len(waits) - keep_n:]
                    for w in extra:
                        nop = mybir.InstNoOp(
                            name=nc.get_next_instruction_name(), ins=[], outs=[]
                        )
                        nop.engine = inst.engine
                        nop.sync_info = bass_rust.SyncInfo(on_wait=[w], on_update=[])
                        new_insts.append(nop)
                    inst.sync_info = bass_rust.SyncInfo(
                        on_wait=keep,
                        on_update=list(si.on_update) if si.on_update else [],
                    )
                new_insts.append(inst)
            if changed:
                blk.instructions = new_insts
    return nc


def build_nc():
    nc = bass.Bass("TRN2", target_bir_lowering=False, debug=False)

    dsr = nc.dram_tensor("dsr", [BPC, DS_LEN], bf16, kind="ExternalInput")
    dsi = nc.dram_tensor("dsi", [BPC, DS_LEN], bf16, kind="ExternalInput")
    scols = nc.dram_tensor("scols", [BPC, 128, 16], f32, kind="ExternalInput")
    tabs = {}
    for r in range(4):
        for form in "AB":
            for h in range(2):
                nm = f"t{form}{r}{h}"
                tabs[(form, r, h)] = nc.dram_tensor(nm, [128, 512], bf16, kind="ExternalInput")
    out = nc.dram_tensor("out", [BPC, K, N], f32, kind="ExternalOutput")

    with tile.TileContext(nc) as tc:
        with (
            tc.tile_pool(name="const", bufs=1) as constp,
            tc.tile_pool(name="win", bufs=2) as winp,
            tc.tile_pool(name="sm", bufs=2) as smp,
            tc.tile_pool(name="u", bufs=2) as up,
            tc.tile_pool(name="pq", bufs=2) as pqp,
            tc.tile_pool(name="bb", bufs=2) as bbp,
            tc.tile_pool(name="sq", bufs=2) as sqp,
            tc.tile_pool(name="chi", bufs=2) as chip,
            tc.tile_pool(name="ps", bufs=2, space="PSUM") as psp,
        ):
            TT = {}
            engs = [nc.sync, nc.scalar, nc.gpsimd]
            for i, (key, dt_) in enumerate(tabs.items()):
                t = constp.tile([128, 512], bf16, tag=f"tab{i}", name=f"tab{i}")
                TT[key] = t
                engs[i % 3].dma_start(t[:], dt_[:])

            def emit_load(b):
                s = {"b": b}
                Tsr = winp.tile([128, 1536], bf16, tag="tsr", name=f"tsr{b}")
                Tsi = winp.tile([128, 1536], bf16, tag="tsi", name=f"tsi{b}")
                nc.sync.dma_start(Tsr[:], bass.AP(dsr, b * DS_LEN + 385, [[1, 128], [1, 1536]]))
                nc.scalar.dma_start(Tsi[:], bass.AP(dsi, b * DS_LEN + 385, [[1, 128], [1, 1536]]))
                scol = smp.tile([128, 16], f32, tag="scol", name=f"scol{b}")
                nc.gpsimd.dma_start(scol[:], scols[b])
                s["T"] = (Tsr, Tsi)
                s["scol"] = scol
                s["u"] = {}
                s["B"] = {}
                return s

            def win(T, j, lo, n):
                ap = T[:]
                return bass.AP(ap.tensor, ap.offset + 639 + 128 * j - lo, [ap.ap[0], [-1, n]])

            def emit_ubuild(s, js, lo, hi):
                Tsr, Tsi = s["T"]
                scol = s["scol"]
                n = hi - lo
                for j in js:
                    w_sr = win(Tsr, j, lo, n)
                    w_si = win(Tsi, j, lo, n)
                    sr_c = scol[:, j:j + 1]
                    si_c = scol[:, 8 + j:9 + j]
                    if lo == 0:
                        ure = up.tile([128, K], bf16, tag=f"u{j}r", name=f"u{j}r_{s['b']}")
                        uim = up.tile([128, K], bf16, tag=f"u{j}i", name=f"u{j}i_{s['b']}")
                        s["u"][j] = (ure, uim)
                    else:
                        ure, uim = s["u"][j]
                    a = pqp.tile([128, K], bf16, tag="ta", name=f"ta{s['b']}{j}{lo}")
                    nc.vector.tensor_scalar_mul(a[:, lo:hi], w_si, si_c)
                    nc.vector.scalar_tensor_tensor(
                        ure[:, lo:hi], w_sr, sr_c, a[:, lo:hi], op0=ALU.mult, op1=ALU.add
                    )
                    b2 = pqp.tile([128, K], bf16, tag="tb", name=f"tb{s['b']}{j}{lo}")
                    nc.vector.tensor_scalar_mul(b2[:, lo:hi], w_si, sr_c)
                    nc.vector.scalar_tensor_tensor(
                        uim[:, lo:hi], w_sr, si_c, b2[:, lo:hi], op0=ALU.mult, op1=ALU.subtract
                    )

            def emit_fft4(s, h, lo, hi):
                # B_r[h-chunk] = sum_c (-i)^{cr} u_{j=2c+h}
                b = s["b"]
                u0r, u0i = s["u"][h]
                u1r, u1i = s["u"][2 + h]
                u2r, u2i = s["u"][4 + h]
                u3r, u3i = s["u"][6 + h]
                if lo == 0:
                    t = {}
                    for nm in ("Pr", "Pi", "Qr", "Qi", "Ur", "Ui", "Vr", "Vi"):
                        t[nm] = pqp.tile([128, K], bf16, tag=f"{nm}{h}", name=f"{nm}{h}_{b}")
                    s[f"t{h}"] = t
                    Bt = {}
                    for r in range(4):
                        for c in "ri":
                            Bt[(r, c)] = bbp.tile(
                                [128, K], bf16, tag=f"b{r}{c}{h}", name=f"b{r}{c}{h}_{b}"
                            )
                    s["B"][h] = Bt
                else:
                    t = s[f"t{h}"]
                    Bt = s["B"][h]
                sl = slice(lo, hi)
                tt = nc.vector.tensor_tensor
                tt(t["Pr"][:, sl], u0r[:, sl], u2r[:, sl], op=ALU.add)
                tt(t["Qr"][:, sl], u0r[:, sl], u2r[:, sl], op=ALU.subtract)
                tt(t["Pi"][:, sl], u0i[:, sl], u2i[:, sl], op=ALU.add)
                tt(t["Qi"][:, sl], u0i[:, sl], u2i[:, sl], op=ALU.subtract)
                tt(t["Ur"][:, sl], u1r[:, sl], u3r[:, sl], op=ALU.add)
                tt(t["Vr"][:, sl], u1r[:, sl], u3r[:, sl], op=ALU.subtract)
                tt(t["Ui"][:, sl], u1i[:, sl], u3i[:, sl], op=ALU.add)
                tt(t["Vi"][:, sl], u1i[:, sl], u3i[:, sl], op=ALU.subtract)
                tt(Bt[(0, "r")][:, sl], t["Pr"][:, sl], t["Ur"][:, sl], op=ALU.add)
                tt(Bt[(0, "i")][:, sl], t["Pi"][:, sl], t["Ui"][:, sl], op=ALU.add)
                tt(Bt[(2, "r")][:, sl], t["Pr"][:, sl], t["Ur"][:, sl], op=ALU.subtract)
                tt(Bt[(2, "i")][:, sl], t["Pi"][:, sl], t["Ui"][:, sl], op=ALU.subtract)
                tt(Bt[(1, "r")][:, sl], t["Qr"][:, sl], t["Vi"][:, sl], op=ALU.add)
                tt(Bt[(1, "i")][:, sl], t["Qi"][:, sl], t["Vr"][:, sl], op=ALU.subtract)
                tt(Bt[(3, "r")][:, sl], t["Qr"][:, sl], t["Vi"][:, sl], op=ALU.subtract)
                tt(Bt[(3, "i")][:, sl], t["Qi"][:, sl], t["Vr"][:, sl], op=ALU.add)

            def emit_kblock(s, kb):
                b = s["b"]
                c0 = 128 * kb
                chi_t = chip.tile([128, N], f32, tag=f"chi{kb % 2}", name=f"chi{b}{kb}")
                for r in range(4):
                    ps = psp.tile([128, 512], f32, tag=f"ps{r}", name=f"ps{b}{kb}{r}")
                    first = True
                    for h in range(2):
                        st = s["B"][h][(r, "r")][:, c0:c0 + 128]
                        nc.tensor.matmul(ps[:], st, TT[("A", r, h)][:], start=first, stop=False)
                        first = False
                    for h in range(2):
                        st = s["B"][h][(r, "i")][:, c0:c0 + 128]
                        nc.tensor.matmul(ps[:], st, TT[("B", r, h)][:], start=False, stop=(h == 1))
                    sq = sqp.tile([128, 512], f32, tag=f"sq{r}", name=f"sq{b}{kb}{r}")
                    nc.scalar.square(sq[:], ps[:])
                    cap = chi_t[:]
                    strided = bass.AP(cap.tensor, cap.offset + r, [cap.ap[0], [4, 256]])
                    eng = nc.vector if r < 2 else nc.gpsimd
                    eng.tensor_tensor(strided, sq[:, 0:256], sq[:, 256:512], op=ALU.add)
                return chi_t

            def emit_store(s, kb, chi_t):
                b = s["b"]
                eng = nc.sync if kb % 2 == 0 else nc.scalar
                eng.dma_start(out[b, 128 * kb:128 * kb + 128, :], chi_t[:])

            # ---- schedule ----
            s0 = emit_load(0)
            emit_ubuild(s0, range(8), 0, 256)
            emit_fft4(s0, 0, 0, 256)
            emit_fft4(s0, 1, 0, 256)
            s1 = emit_load(1)
            emit_ubuild(s0, range(8), 256, 512)
            c00 = emit_kblock(s0, 0)
            emit_fft4(s0, 0, 256, 512)
            emit_fft4(s0, 1, 256, 512)
            emit_store(s0, 0, c00)
            c01 = emit_kblock(s0, 1)
            emit_ubuild(s1, range(4), 0, 256)
            emit_store(s0, 1, c01)
            c02 = emit_kblock(s0, 2)
            emit_ubuild(s1, range(4, 8), 0, 256)
            emit_store(s0, 2, c02)
            c03 = emit_kblock(s0, 3)
            emit_fft4(s1, 0, 0, 256)
            emit_fft4(s1, 1, 0, 256)
            emit_store(s0, 3, c03)
            emit_ubuild(s1, range(8), 256, 512)
            c10 = emit_kblock(s1, 0)
            emit_fft4(s1, 0, 256, 512)
            emit_store(s1, 0, c10)
            c11 = emit_kblock(s1, 1)
            emit_fft4(s1, 1, 256, 512)
            emit_store(s1, 1, c11)
            c12 = emit_kblock(s1, 2)
            emit_store(s1, 2, c12)
            c13 = emit_kblock(s1, 3)
            emit_store(s1, 3, c13)

    _split_excess_waits(nc)
    return nc


_NC_CACHE = {}


def _get_nc():
    if "nc" not in _NC_CACHE:
        _NC_CACHE["nc"] = build_nc()
    return _NC_CACHE["nc"]


def _get_tables():
    if "tabs" not in _NC_CACHE:
        mpp = np.arange(256, dtype=np.float64)[:, None]
        t = np.arange(256, dtype=np.float64)[None, :]
        t_sh = (t + 128) % 256
        tabs = {}
        for r in range(4):
            ang = 2.0 * np.pi * ((mpp * (r + 4 * t_sh)) % 1024) / 1024
            Mc = np.cos(ang)
            Ms = np.sin(ang)
            for h in range(2):
                sl = slice(128 * h, 128 * h + 128)
                tabs[f"tA{r}{h}"] = np.concatenate(
                    [Mc[sl], -Ms[sl]], axis=1
                ).astype(ml_dtypes.bfloat16)
                tabs[f"tB{r}{h}"] = np.concatenate(
                    [Ms[sl], Mc[sl]], axis=1
                ).astype(ml_dtypes.bfloat16)
        _NC_CACHE["tabs"] = tabs
    return _NC_CACHE["tabs"]


def _host_prep(sr, si):
    """Per-core input prep. sr/si: [BPC, N] float32 (already prescaled)."""
    dsr = np.tile(sr, (1, 3))[:, :DS_LEN].astype(ml_dtypes.bfloat16)
    dsi = np.tile(si, (1, 3))[:, :DS_LEN].astype(ml_dtypes.bfloat16)
    scols = np.concatenate(
        [
            sr.reshape(BPC, 8, 128).transpose(0, 2, 1),
            si.reshape(BPC, 8, 128).transpose(0, 2, 1),
        ],
        axis=2,
    ).astype(np.float32).copy()
    im = {"dsr": dsr, "dsi": dsi, "scols": scols}
    im.update(_get_tables())
    return im


def kernel(s_real: np.ndarray, s_imag: np.ndarray) -> np.ndarray:
    s_real = np.asarray(s_real, dtype=np.float32)
    s_imag = np.asarray(s_imag, dtype=np.float32)
    # exact normalization: max chi = (sum |s|^2)^2 (Cauchy-Schwarz, attained
    # at k=0,f=0), so prescale s by (sum|s|^2)^{-1/2}
    pw = (s_real.astype(np.float64) ** 2 + s_imag.astype(np.float64) ** 2).sum(
        axis=1, keepdims=True
    )
    g = 1.0 / np.sqrt(pw)
    sr_s = (s_real * g).astype(np.float32)
    si_s = (s_imag * g).astype(np.float32)

    nc = _get_nc()
    in_maps = [
        _host_prep(sr_s[c * BPC:(c + 1) * BPC], si_s[c * BPC:(c + 1) * BPC])
        for c in range(NCORES)
    ]
    res = bass_utils.run_bass_kernel_spmd(nc, in_maps, core_ids=list(range(NCORES)))
    chi = np.concatenate([r["out"] for r in res.results], axis=0)  # [B, 512, N]

    full = np.empty((B, N, N), dtype=np.float32)
    full[:, 512:1024, :] = chi
    # mirror: rows r in [1,512): chi[r] = flip_f(chi_direct[512 - r])
    src = chi[:, 511:0:-1, :]                      # k2 = 511..1 -> rows 1..511
    full[:, 1:512, 0] = src[:, :, 0]
    full[:, 1:512, 1:] = src[:, :, :0:-1]
    # row 0 (k=512) on host in float64
    s64 = (sr_s.astype(np.float64) + 1j * si_s.astype(np.float64))
    r512 = s64 * np.conj(np.roll(s64, 512, axis=1))
    x512 = np.fft.fft(r512, axis=1)
    full[:, 0, :] = np.fft.fftshift(
        (x512 * np.conj(x512)).real, axes=-1
    ).astype(np.float32)
    return full


# revision 3
# speedup vs baseline: 1.2056x; 1.0237x over previous
"""Radix-4 DIF ambiguity kernel.

Per batch: u_c = s[m]*conj(s[m-k]) sliding-window products (DVE, bf16),
FFT4 combine over c (DVE, bf16), then 4 branch DFT-256 matmuls with
re/im-concatenated bf16 tables (PE, 512-wide moving), |X|^2 via ACT squares
+ DVE/Pool pair adds. Normalization is exact-by-construction (Cauchy-Schwarz:
max chi = (sum|s|^2)^2) and folded into a host prescale of s. Only k in
[0,512) is computed on device; row k=512 and the mirror half-plane
chi[k,f] = chi[N-k, -f] are assembled during host-side unsharding.
"""

import numpy as np
import ml_dtypes

import bass_rust
import concourse.bass as bass
import concourse.mybir as mybir
import concourse.tile as tile
import concourse.bass_utils as bass_utils

B, N = 16, 1024
NCORES = 8
BPC = B // NCORES
K = 512
DS_LEN = 2176

f32 = mybir.dt.float32
bf16 = mybir.dt.bfloat16
ALU = mybir.AluOpType


def _split_excess_waits(nc):
    for f in nc.m.functions:
        for blk in f.blocks:
            insts = list(blk.instructions)
            new_insts = []
            changed = False
            for inst in insts:
                si = inst.sync_info
                waits = list(si.on_wait) if (si is not None and si.on_wait) else []
                keep_n = 0 if isinstance(inst, mybir.InstDrain) else 1
                if len(waits) > keep_n:
                    changed = True
                    extra = waits[: len(waits) - keep_n]
                    keep = waits[len(waits) - keep_n:]
                    for w in extra:
                        nop = mybir.InstNoOp(
                            name=nc.get_next_instruction_name(), ins=[], outs=[]
                        )
                        nop.engine = inst.engine
                        nop.sync_info = bass_rust.SyncInfo(on_wait=[w], on_update=[])
                        new_insts.append(nop)
                    inst.sync_info = bass_rust.SyncInfo(
                        on_wait=keep,
                        on_update=list(si.on_update) if si.on_update else [],
                    )
                new_insts.append(inst)
            if changed:
                blk.instructions = new_insts
    return nc


def build_nc():
    nc = bass.Bass("TRN2", target_bir_lowering=False, debug=False)

    dsr = nc.dram_tensor("dsr", [BPC, DS_LEN], bf16, kind="ExternalInput")
    dsi = nc.dram_tensor("dsi", [BPC, DS_LEN], bf16, kind="ExternalInput")
    scols = nc.dram_tensor("scols", [BPC, 128, 16], f32, kind="ExternalInput")
    tabs = {}
    for r in range(4):
        for form in "AB":
            for h in range(2):
                nm = f"t{form}{r}{h}"
                tabs[(form, r, h)] = nc.dram_tensor(nm, [128, 512], bf16, kind="ExternalInput")
    out = nc.dram_tensor("out", [BPC, K, N], f32, kind="ExternalOutput")

    with tile.TileContext(nc) as tc:
        with (
            tc.tile_pool(name="const", bufs=1) as constp,
            tc.tile_pool(name="win", bufs=2) as winp,
            tc.tile_pool(name="sm", bufs=2) as smp,
            tc.tile_pool(name="u", bufs=2) as up,
            tc.tile_pool(name="pq", bufs=2) as pqp,
            tc.tile_pool(name="bb", bufs=2) as bbp,
            tc.tile_pool(name="sq", bufs=2) as sqp,
            tc.tile_pool(name="chi", bufs=2) as chip,
            tc.tile_pool(name="ps", bufs=2, space="PSUM") as psp,
        ):
            TT = {}
            for i, key in enumerate(tabs):
                TT[key] = constp.tile([128, 512], bf16, tag=f"tab{i}", name=f"tab{i}")

            def load_tables():
                engs = [nc.sync, nc.scalar, nc.gpsimd]
                for i, (key, dt_) in enumerate(tabs.items()):
                    engs[i % 3].dma_start(TT[key][:], dt_[:])

            def emit_load(b):
                s = {"b": b}
                Tsr = winp.tile([128, 1536], bf16, tag="tsr", name=f"tsr{b}")
                Tsi = winp.tile([128, 1536], bf16, tag="tsi", name=f"tsi{b}")
                nc.sync.dma_start(Tsr[:], bass.AP(dsr, b * DS_LEN + 385, [[1, 128], [1, 1536]]))
                nc.scalar.dma_start(Tsi[:], bass.AP(dsi, b * DS_LEN + 385, [[1, 128], [1, 1536]]))
                scol = smp.tile([128, 16], f32, tag="scol", name=f"scol{b}")
                nc.gpsimd.dma_start(scol[:], scols[b])
                s["T"] = (Tsr, Tsi)
                s["scol"] = scol
                s["u"] = {}
                s["B"] = {}
                return s

            def win(T, j, lo, n):
                ap = T[:]
                return bass.AP(ap.tensor, ap.offset + 639 + 128 * j - lo, [ap.ap[0], [-1, n]])

            def emit_ubuild(s, js, lo, hi):
                Tsr, Tsi = s["T"]
                scol = s["scol"]
                n = hi - lo
                for j in js:
                    w_sr = win(Tsr, j, lo, n)
                    w_si = win(Tsi, j, lo, n)
                    sr_c = scol[:, j:j + 1]
                    si_c = scol[:, 8 + j:9 + j]
                    if lo == 0:
                        ure = up.tile([128, K], bf16, tag=f"u{j}r", name=f"u{j}r_{s['b']}")
                        uim = up.tile([128, K], bf16, tag=f"u{j}i", name=f"u{j}i_{s['b']}")
                        s["u"][j] = (ure, uim)
                    else:
                        ure, uim = s["u"][j]
                    a = pqp.tile([128, K], bf16, tag="ta", name=f"ta{s['b']}{j}{lo}")
                    nc.vector.tensor_scalar_mul(a[:, lo:hi], w_si, si_c)
                    nc.vector.scalar_tensor_tensor(
                        ure[:, lo:hi], w_sr, sr_c, a[:, lo:hi], op0=ALU.mult, op1=ALU.add
                    )
                    b2 = pqp.tile([128, K], bf16, tag="tb", name=f"tb{s['b']}{j}{lo}")
                    nc.vector.tensor_scalar_mul(b2[:, lo:hi], w_si, sr_c)
                    nc.vector.scalar_tensor_tensor(
                        uim[:, lo:hi], w_sr, si_c, b2[:, lo:hi], op0=ALU.mult, op1=ALU.subtract
                    )

            def emit_fft4(s, h, lo, hi):
                # B_r[h-chunk] = sum_c (-i)^{cr} u_{j=2c+h}
                b = s["b"]
                u0r, u0i = s["u"][h]
                u1r, u1i = s["u"][2 + h]
                u2r, u2i = s["u"][4 + h]
                u3r, u3i = s["u"][6 + h]
                if lo == 0:
                    t = {}
                    for nm in ("Pr", "Pi", "Qr", "Qi", "Ur", "Ui", "Vr", "Vi"):
                        t[nm] = pqp.tile([128, K], bf16, tag=f"{nm}{h}", name=f"{nm}{h}_{b}")
                    s[f"t{h}"] = t
                    Bt = {}
                    for r in range(4):
                        for c in "ri":
                            Bt[(r, c)] = bbp.tile(
                                [128, K], bf16, tag=f"b{r}{c}{h}", name=f"b{r}{c}{h}_{b}"
                            )
                    s["B"][h] = Bt
                else:
                    t = s[f"t{h}"]
                    Bt = s["B"][h]
                sl = slice(lo, hi)
                tt = nc.vector.tensor_tensor
                tt(t["Pr"][:, sl], u0r[:, sl], u2r[:, sl], op=ALU.add)
                tt(t["Qr"][:, sl], u0r[:, sl], u2r[:, sl], op=ALU.subtract)
                tt(t["Pi"][:, sl], u0i[:, sl], u2i[:, sl], op=ALU.add)
                tt(t["Qi"][:, sl], u0i[:, sl], u2i[:, sl], op=ALU.subtract)
                tt(t["Ur"][:, sl], u1r[:, sl], u3r[:, sl], op=ALU.add)
                tt(t["Vr"][:, sl], u1r[:, sl], u3r[:, sl], op=ALU.subtract)
                tt(t["Ui"][:, sl], u1i[:, sl], u3i[:, sl], op=ALU.add)
                tt(t["Vi"][:, sl], u1i[:, sl], u3i[:, sl], op=ALU.subtract)
                tt(Bt[(0, "r")][:, sl], t["Pr"][:, sl], t["Ur"][:, sl], op=ALU.add)
                tt(Bt[(0, "i")][:, sl], t["Pi"][:, sl], t["Ui"][:, sl], op=ALU.add)
                tt(Bt[(2, "r")][:, sl], t["Pr"][:, sl], t["Ur"][:, sl], op=ALU.subtract)
                tt(Bt[(2, "i")][:, sl], t["Pi"][:, sl], t["Ui"][:, sl], op=ALU.subtract)
                tt(Bt[(1, "r")][:, sl], t["Qr"][:, sl], t["Vi"][:, sl], op=ALU.add)
                tt(Bt[(1, "i")][:, sl], t["Qi"][:, sl], t["Vr"][:, sl], op=ALU.subtract)
                tt(Bt[(3, "r")][:, sl], t["Qr"][:, sl], t["Vi"][:, sl], op=ALU.subtract)
                tt(Bt[(3, "i")][:, sl], t["Qi"][:, sl], t["Vr"][:, sl], op=ALU.add)

            def emit_kblock(s, kb):
                b = s["b"]
                c0 = 128 * kb
                chi_t = chip.tile([128, N], f32, tag=f"chi{kb % 2}", name=f"chi{b}{kb}")
                for r in range(4):
                    ps = psp.tile([128, 512], f32, tag=f"ps{r}", name=f"ps{b}{kb}{r}")
                    first = True
                    for h in range(2):
                        st = s["B"][h][(r, "r")][:, c0:c0 + 128]
                        nc.tensor.matmul(ps[:], st, TT[("A", r, h)][:], start=first, stop=False)
                        first = False
                    for h in range(2):
                        st = s["B"][h][(r, "i")][:, c0:c0 + 128]
                        nc.tensor.matmul(ps[:], st, TT[("B", r, h)][:], start=False, stop=(h == 1))
                    sq = sqp.tile([128, 512], f32, tag=f"sq{r}", name=f"sq{b}{kb}{r}")
                    nc.scalar.square(sq[:], ps[:])
                    cap = chi_t[:]
                    strided = bass.AP(cap.tensor, cap.offset + r, [cap.ap[0], [4, 256]])
                    eng = nc.gpsimd
                    eng.tensor_tensor(strided, sq[:, 0:256], sq[:, 256:512], op=ALU.add)
                return chi_t

            def emit_store(s, kb, chi_t):
                b = s["b"]
                eng = nc.sync if kb % 2 == 0 else nc.scalar
                eng.dma_start(out[b, 128 * kb:128 * kb + 128, :], chi_t[:])

            # ---- schedule ----
            s0 = emit_load(0)
            emit_ubuild(s0, range(8), 0, 512)
            emit_fft4(s0, 0, 0, 512)
            emit_fft4(s0, 1, 0, 512)
            s1 = emit_load(1)
            load_tables()
            c00 = emit_kblock(s0, 0)
            emit_ubuild(s1, range(3), 0, 512)
            emit_store(s0, 0, c00)
            c01 = emit_kblock(s0, 1)
            emit_ubuild(s1, range(3, 6), 0, 512)
            emit_store(s0, 1, c01)
            c02 = emit_kblock(s0, 2)
            emit_ubuild(s1, range(6, 8), 0, 512)
            emit_fft4(s1, 0, 0, 512)
            emit_store(s0, 2, c02)
            c03 = emit_kblock(s0, 3)
            emit_fft4(s1, 1, 0, 512)
            emit_store(s0, 3, c03)
            c10 = emit_kblock(s1, 0)
            emit_store(s1, 0, c10)
            c11 = emit_kblock(s1, 1)
            emit_store(s1, 1, c11)
            c12 = emit_kblock(s1, 2)
            emit_store(s1, 2, c12)
            c13 = emit_kblock(s1, 3)
            emit_store(s1, 3, c13)

    _split_excess_waits(nc)
    return nc


_NC_CACHE = {}


def _get_nc():
    if "nc" not in _NC_CACHE:
        _NC_CACHE["nc"] = build_nc()
    return _NC_CACHE["nc"]


def _get_tables():
    if "tabs" not in _NC_CACHE:
        mpp = np.arange(256, dtype=np.float64)[:, None]
        t = np.arange(256, dtype=np.float64)[None, :]
        t_sh = (t + 128) % 256
        tabs = {}
        for r in range(4):
            ang = 2.0 * np.pi * ((mpp * (r + 4 * t_sh)) % 1024) / 1024
            Mc = np.cos(ang)
            Ms = np.sin(ang)
            for h in range(2):
                sl = slice(128 * h, 128 * h + 128)
                tabs[f"tA{r}{h}"] = np.concatenate(
                    [Mc[sl], -Ms[sl]], axis=1
                ).astype(ml_dtypes.bfloat16)
                tabs[f"tB{r}{h}"] = np.concatenate(
                    [Ms[sl], Mc[sl]], axis=1
                ).astype(ml_dtypes.bfloat16)
        _NC_CACHE["tabs"] = tabs
    return _NC_CACHE["tabs"]


def _host_prep(sr, si):
    """Per-core input prep. sr/si: [BPC, N] float32 (already prescaled)."""
    dsr = np.tile(sr, (1, 3))[:, :DS_LEN].astype(ml_dtypes.bfloat16)
    dsi = np.tile(si, (1, 3))[:, :DS_LEN].astype(ml_dtypes.bfloat16)
    scols = np.concatenate(
        [
            sr.reshape(BPC, 8, 128).transpose(0, 2, 1),
            si.reshape(BPC, 8, 128).transpose(0, 2, 1),
        ],
        axis=2,
    ).astype(np.float32).copy()
    im = {"dsr": dsr, "dsi": dsi, "scols": scols}
    im.update(_get_tables())
    return im


def kernel(s_real: np.ndarray, s_imag: np.ndarray) -> np.ndarray:
    s_real = np.asarray(s_real, dtype=np.float32)
    s_imag = np.asarray(s_imag, dtype=np.float32)
    # exact normalization: max chi = (sum |s|^2)^2 (Cauchy-Schwarz, attained
    # at k=0,f=0), so prescale s by (sum|s|^2)^{-1/2}
    pw = (s_real.astype(np.float64) ** 2 + s_imag.astype(np.float64) ** 2).sum(
        axis=1, keepdims=True
    )
    g = 1.0 / np.sqrt(pw)
    sr_s = (s_real * g).astype(np.float32)
    si_s = (s_imag * g).astype(np.float32)

    nc = _get_nc()
    in_maps = [
        _host_prep(sr_s[c * BPC:(c + 1) * BPC], si_s[c * BPC:(c + 1) * BPC])
        for c in range(NCORES)
    ]
    res = bass_utils.run_bass_kernel_spmd(nc, in_maps, core_ids=list(range(NCORES)))
    chi = np.concatenate([r["out"] for r in res.results], axis=0)  # [B, 512, N]

    full = np.empty((B, N, N), dtype=np.float32)
    full[:, 512:1024, :] = chi
    # mirror: rows r in [1,512): chi[r] = flip_f(chi_direct[512 - r])
    src = chi[:, 511:0:-1, :]                      # k2 = 511..1 -> rows 1..511
    full[:, 1:512, 0] = src[:, :, 0]
    full[:, 1:512, 1:] = src[:, :, :0:-1]
    # row 0 (k=512) on host in float64
    s64 = (sr_s.astype(np.float64) + 1j * si_s.astype(np.float64))
    r512 = s64 * np.conj(np.roll(s64, 512, axis=1))
    x512 = np.fft.fft(r512, axis=1)
    full[:, 0, :] = np.fft.fftshift(
        (x512 * np.conj(x512)).real, axes=-1
    ).astype(np.float32)
    return full


# revision 4
# speedup vs baseline: 1.2512x; 1.0378x over previous
"""Radix-4 DIF ambiguity kernel.

Per batch: u_c = s[m]*conj(s[m-k]) sliding-window products (DVE, bf16),
FFT4 combine over c (DVE, bf16), then 4 branch DFT-256 matmuls with
re/im-concatenated bf16 tables (PE, 512-wide moving), |X|^2 via ACT squares
+ DVE/Pool pair adds. Normalization is exact-by-construction (Cauchy-Schwarz:
max chi = (sum|s|^2)^2) and folded into a host prescale of s. Only k in
[0,512) is computed on device; row k=512 and the mirror half-plane
chi[k,f] = chi[N-k, -f] are assembled during host-side unsharding.
"""

import numpy as np
import ml_dtypes

import bass_rust
import concourse.bass as bass
import concourse.mybir as mybir
import concourse.tile as tile
import concourse.bass_utils as bass_utils

B, N = 16, 1024
NCORES = 8
BPC = B // NCORES
K = 512
DS_LEN = 2176

f32 = mybir.dt.float32
bf16 = mybir.dt.bfloat16
ALU = mybir.AluOpType


def _split_excess_waits(nc):
    for f in nc.m.functions:
        for blk in f.blocks:
            insts = list(blk.instructions)
            new_insts = []
            changed = False
            for inst in insts:
                si = inst.sync_info
                waits = list(si.on_wait) if (si is not None and si.on_wait) else []
                keep_n = 0 if isinstance(inst, mybir.InstDrain) else 1
                if len(waits) > keep_n:
                    changed = True
                    extra = waits[: len(waits) - keep_n]
                    keep = waits[len(waits) - keep_n:]
                    for w in extra:
                        nop = mybir.InstNoOp(
                            name=nc.get_next_instruction_name(), ins=[], outs=[]
                        )
                        nop.engine = inst.engine
                        nop.sync_info = bass_rust.SyncInfo(on_wait=[w], on_update=[])
                        new_insts.append(nop)
                    inst.sync_info = bass_rust.SyncInfo(
                        on_wait=keep,
                        on_update=list(si.on_update) if si.on_update else [],
                    )
                new_insts.append(inst)
            if changed:
                blk.instructions = new_insts
    return nc


def build_nc():
    nc = bass.Bass("TRN2", target_bir_lowering=False, debug=False)

    dsr = nc.dram_tensor("dsr", [BPC, DS_LEN], bf16, kind="ExternalInput")
    dsi = nc.dram_tensor("dsi", [BPC, DS_LEN], bf16, kind="ExternalInput")
    scols = nc.dram_tensor("scols", [BPC, 128, 16], f32, kind="ExternalInput")
    tabs = {}
    for r in range(4):
        for form in "AB":
            for h in range(2):
                nm = f"t{form}{r}{h}"
                tabs[(form, r, h)] = nc.dram_tensor(nm, [128, 512], bf16, kind="ExternalInput")
    out = nc.dram_tensor("out", [BPC, K, N], f32, kind="ExternalOutput")

    with tile.TileContext(nc) as tc:
        with (
            tc.tile_pool(name="const", bufs=1) as constp,
            tc.tile_pool(name="win", bufs=2) as winp,
            tc.tile_pool(name="sm", bufs=2) as smp,
            tc.tile_pool(name="u", bufs=2) as up,
            tc.tile_pool(name="pq", bufs=2) as pqp,
            tc.tile_pool(name="bb", bufs=2) as bbp,
            tc.tile_pool(name="sq", bufs=2) as sqp,
            tc.tile_pool(name="chi", bufs=2) as chip,
            tc.tile_pool(name="ps", bufs=2, space="PSUM") as psp,
        ):
            TT = {}
            for i, key in enumerate(tabs):
                TT[key] = constp.tile([128, 512], bf16, tag=f"tab{i}", name=f"tab{i}")

            def load_tables():
                engs = [nc.sync, nc.gpsimd]
                for i, (key, dt_) in enumerate(tabs.items()):
                    engs[i % 2].dma_start(TT[key][:], dt_[:])

            def emit_load(b):
                s = {"b": b}
                Tsr = winp.tile([128, 1536], bf16, tag="tsr", name=f"tsr{b}")
                Tsi = winp.tile([128, 1536], bf16, tag="tsi", name=f"tsi{b}")
                for p0, p1 in ((0, 64), (64, 128)):
                    nc.sync.dma_start(
                        Tsr[p0:p1, :],
                        bass.AP(dsr, b * DS_LEN + 385 + p0, [[1, p1 - p0], [1, 1536]]),
                    )
                    nc.gpsimd.dma_start(
                        Tsi[p0:p1, :],
                        bass.AP(dsi, b * DS_LEN + 385 + p0, [[1, p1 - p0], [1, 1536]]),
                    )
                scol = smp.tile([128, 16], f32, tag="scol", name=f"scol{b}")
                nc.sync.dma_start(scol[:], scols[b])
                s["T"] = (Tsr, Tsi)
                s["scol"] = scol
                s["u"] = {}
                s["B"] = {}
                return s

            def win(T, j, lo, n):
                ap = T[:]
                return bass.AP(ap.tensor, ap.offset + 639 + 128 * j - lo, [ap.ap[0], [-1, n]])

            def emit_ubuild(s, js, lo, hi):
                Tsr, Tsi = s["T"]
                scol = s["scol"]
                n = hi - lo
                for j in js:
                    w_sr = win(Tsr, j, lo, n)
                    w_si = win(Tsi, j, lo, n)
                    sr_c = scol[:, j:j + 1]
                    si_c = scol[:, 8 + j:9 + j]
                    if lo == 0:
                        ut = up.tile([128, 2 * K], bf16, tag=f"u{j}", name=f"u{j}_{s['b']}")
                        s["u"][j] = ut
                    else:
                        ut = s["u"][j]
                    ure = ut[:, lo:hi]
                    uim = ut[:, K + lo:K + hi]
                    a = pqp.tile([128, K], bf16, tag="ta", name=f"ta{s['b']}{j}{lo}")
                    nc.vector.tensor_scalar_mul(a[:, lo:hi], w_si, si_c)
                    nc.vector.scalar_tensor_tensor(
                        ure, w_sr, sr_c, a[:, lo:hi], op0=ALU.mult, op1=ALU.add
                    )
                    b2 = pqp.tile([128, K], bf16, tag="tb", name=f"tb{s['b']}{j}{lo}")
                    nc.vector.tensor_scalar_mul(b2[:, lo:hi], w_si, sr_c)
                    nc.vector.scalar_tensor_tensor(
                        uim, w_sr, si_c, b2[:, lo:hi], op0=ALU.mult, op1=ALU.subtract
                    )

            def emit_fft4(s, h, lo, hi):
                # B_r[h-chunk] = sum_c (-i)^{cr} u_{j=2c+h}; tiles pack (re|im)
                b = s["b"]
                u0 = s["u"][h]
                u1 = s["u"][2 + h]
                u2 = s["u"][4 + h]
                u3 = s["u"][6 + h]
                t = {}
                for nm in ("P", "Q", "U", "W"):
                    t[nm] = pqp.tile([128, 2 * K], bf16, tag=f"{nm}{h}", name=f"{nm}{h}_{b}")
                Bt = {}
                for r in range(4):
                    Bt[r] = bbp.tile([128, 2 * K], bf16, tag=f"b{r}{h}", name=f"b{r}{h}_{b}")
                s["B"][h] = Bt
                tt = nc.vector.tensor_tensor
                tt(t["P"][:], u0[:], u2[:], op=ALU.add)
                tt(t["Q"][:], u0[:], u2[:], op=ALU.subtract)
                tt(t["U"][:], u1[:], u3[:], op=ALU.add)
                # W = (V.im | -V.re) where V = u1 - u3, so B1 = Q+W, B3 = Q-W
                tt(t["W"][:, 0:K], u1[:, K:2 * K], u3[:, K:2 * K], op=ALU.subtract)
                tt(t["W"][:, K:2 * K], u3[:, 0:K], u1[:, 0:K], op=ALU.subtract)
                tt(Bt[0][:], t["P"][:], t["U"][:], op=ALU.add)
                tt(Bt[2][:], t["P"][:], t["U"][:], op=ALU.subtract)
                tt(Bt[1][:], t["Q"][:], t["W"][:], op=ALU.add)
                tt(Bt[3][:], t["Q"][:], t["W"][:], op=ALU.subtract)

            def emit_kblock(s, kb):
                b = s["b"]
                c0 = 128 * kb
                chi_t = chip.tile([128, N], f32, tag=f"chi{kb % 2}", name=f"chi{b}{kb}")
                for r in range(4):
                    ps = psp.tile([128, 512], f32, tag=f"ps{r}", name=f"ps{b}{kb}{r}")
                    first = True
                    for h in range(2):
                        st = s["B"][h][r][:, c0:c0 + 128]
                        nc.tensor.matmul(ps[:], st, TT[("A", r, h)][:], start=first, stop=False)
                        first = False
                    for h in range(2):
                        st = s["B"][h][r][:, K + c0:K + c0 + 128]
                        nc.tensor.matmul(ps[:], st, TT[("B", r, h)][:], start=False, stop=(h == 1))
                    sq = sqp.tile([128, 512], f32, tag=f"sq{r}", name=f"sq{b}{kb}{r}")
                    nc.scalar.square(sq[:], ps[:])
                    cap = chi_t[:]
                    strided = bass.AP(cap.tensor, cap.offset + r, [cap.ap[0], [4, 256]])
                    eng = nc.vector if (b == 1 and kb == 3) else nc.gpsimd
                    eng.tensor_tensor(strided, sq[:, 0:256], sq[:, 256:512], op=ALU.add)
                return chi_t

            def emit_store(s, kb, chi_t):
                b = s["b"]
                eng = nc.sync if kb % 2 == 0 else nc.scalar
                eng.dma_start(out[b, 128 * kb:128 * kb + 128, :], chi_t[:])

            # ---- schedule ----
            s0 = emit_load(0)
            emit_ubuild(s0, range(8), 0, 512)
            emit_fft4(s0, 0, 0, 512)
            emit_fft4(s0, 1, 0, 512)
            s1 = emit_load(1)
            load_tables()
            c00 = emit_kblock(s0, 0)
            emit_ubuild(s1, range(3), 0, 512)
            emit_store(s0, 0, c00)
            c01 = emit_kblock(s0, 1)
            emit_ubuild(s1, range(3, 6), 0, 512)
            emit_store(s0, 1, c01)
            c02 = emit_kblock(s0, 2)
            emit_ubuild(s1, range(6, 8), 0, 512)
            emit_fft4(s1, 0, 0, 512)
            emit_store(s0, 2, c02)
            c03 = emit_kblock(s0, 3)
            emit_fft4(s1, 1, 0, 512)
            emit_store(s0, 3, c03)
            c10 = emit_kblock(s1, 0)
            emit_store(s1, 0, c10)
            c11 = emit_kblock(s1, 1)
            emit_store(s1, 1, c11)
            c12 = emit_kblock(s1, 2)
            emit_store(s1, 2, c12)
            c13 = emit_kblock(s1, 3)
            emit_store(s1, 3, c13)

    _split_excess_waits(nc)
    return nc


_NC_CACHE = {}


def _get_nc():
    if "nc" not in _NC_CACHE:
        _NC_CACHE["nc"] = build_nc()
    return _NC_CACHE["nc"]


def _get_tables():
    if "tabs" not in _NC_CACHE:
        mpp = np.arange(256, dtype=np.float64)[:, None]
        t = np.arange(256, dtype=np.float64)[None, :]
        t_sh = (t + 128) % 256
        tabs = {}
        for r in range(4):
            ang = 2.0 * np.pi * ((mpp * (r + 4 * t_sh)) % 1024) / 1024
            Mc = np.cos(ang)
            Ms = np.sin(ang)
            for h in range(2):
                sl = slice(128 * h, 128 * h + 128)
                tabs[f"tA{r}{h}"] = np.concatenate(
                    [Mc[sl], -Ms[sl]], axis=1
                ).astype(ml_dtypes.bfloat16)
                tabs[f"tB{r}{h}"] = np.concatenate(
                    [Ms[sl], Mc[sl]], axis=1
                ).astype(ml_dtypes.bfloat16)
        _NC_CACHE["tabs"] = tabs
    return _NC_CACHE["tabs"]


def _host_prep(sr, si):
    """Per-core input prep. sr/si: [BPC, N] float32 (already prescaled)."""
    dsr = np.tile(sr, (1, 3))[:, :DS_LEN].astype(ml_dtypes.bfloat16)
    dsi = np.tile(si, (1, 3))[:, :DS_LEN].astype(ml_dtypes.bfloat16)
    scols = np.concatenate(
        [
            sr.reshape(BPC, 8, 128).transpose(0, 2, 1),
            si.reshape(BPC, 8, 128).transpose(0, 2, 1),
        ],
        axis=2,
    ).astype(np.float32).copy()
    im = {"dsr": dsr, "dsi": dsi, "scols": scols}
    im.update(_get_tables())
    return im


def kernel(s_real: np.ndarray, s_imag: np.ndarray) -> np.ndarray:
    s_real = np.asarray(s_real, dtype=np.float32)
    s_imag = np.asarray(s_imag, dtype=np.float32)
    # exact normalization: max chi = (sum |s|^2)^2 (Cauchy-Schwarz, attained
    # at k=0,f=0), so prescale s by (sum|s|^2)^{-1/2}
    pw = (s_real.astype(np.float64) ** 2 + s_imag.astype(np.float64) ** 2).sum(
        axis=1, keepdims=True
    )
    g = 1.0 / np.sqrt(pw)
    sr_s = (s_real * g).astype(np.float32)
    si_s = (s_imag * g).astype(np.float32)

    nc = _get_nc()
    in_maps = [
        _host_prep(sr_s[c * BPC:(c + 1) * BPC], si_s[c * BPC:(c + 1) * BPC])
        for c in range(NCORES)
    ]
    res = bass_utils.run_bass_kernel_spmd(nc, in_maps, core_ids=list(range(NCORES)))
    chi = np.concatenate([r["out"] for r in res.results], axis=0)  # [B, 512, N]

    full = np.empty((B, N, N), dtype=np.float32)
    full[:, 512:1024, :] = chi
    # mirror: rows r in [1,512): chi[r] = flip_f(chi_direct[512 - r])
    src = chi[:, 511:0:-1, :]                      # k2 = 511..1 -> rows 1..511
    full[:, 1:512, 0] = src[:, :, 0]
    full[:, 1:512, 1:] = src[:, :, :0:-1]
    # row 0 (k=512) on host in float64
    s64 = (sr_s.astype(np.float64) + 1j * si_s.astype(np.float64))
    r512 = s64 * np.conj(np.roll(s64, 512, axis=1))
    x512 = np.fft.fft(r512, axis=1)
    full[:, 0, :] = np.fft.fftshift(
        (x512 * np.conj(x512)).real, axes=-1
    ).astype(np.float32)
    return full


# revision 5
# speedup vs baseline: 1.3213x; 1.0561x over previous
"""Radix-4 DIF ambiguity kernel.

Per batch: u_c = s[m]*conj(s[m-k]) sliding-window products (DVE, bf16),
FFT4 combine over c (DVE, bf16), then 4 branch DFT-256 matmuls with
re/im-concatenated bf16 tables (PE, 512-wide moving), |X|^2 via ACT squares
+ DVE/Pool pair adds. Normalization is exact-by-construction (Cauchy-Schwarz:
max chi = (sum|s|^2)^2) and folded into a host prescale of s. Only k in
[0,512) is computed on device; row k=512 and the mirror half-plane
chi[k,f] = chi[N-k, -f] are assembled during host-side unsharding.
"""

import numpy as np
import ml_dtypes

import bass_rust
import concourse.bass as bass
import concourse.mybir as mybir
import concourse.tile as tile
import concourse.bass_utils as bass_utils

B, N = 16, 1024
NCORES = 8
BPC = B // NCORES
K = 512
DS_LEN = 2176

f32 = mybir.dt.float32
bf16 = mybir.dt.bfloat16
ALU = mybir.AluOpType


def _split_excess_waits(nc):
    for f in nc.m.functions:
        for blk in f.blocks:
            insts = list(blk.instructions)
            new_insts = []
            changed = False
            for inst in insts:
                si = inst.sync_info
                waits = list(si.on_wait) if (si is not None and si.on_wait) else []
                keep_n = 0 if isinstance(inst, mybir.InstDrain) else 1
                if len(waits) > keep_n:
                    changed = True
                    extra = waits[: len(waits) - keep_n]
                    keep = waits[len(waits) - keep_n:]
                    for w in extra:
                        nop = mybir.InstNoOp(
                            name=nc.get_next_instruction_name(), ins=[], outs=[]
                        )
                        nop.engine = inst.engine
                        nop.sync_info = bass_rust.SyncInfo(on_wait=[w], on_update=[])
                        new_insts.append(nop)
                    inst.sync_info = bass_rust.SyncInfo(
                        on_wait=keep,
                        on_update=list(si.on_update) if si.on_update else [],
                    )
                new_insts.append(inst)
            if changed:
                blk.instructions = new_insts
    return nc


def build_nc():
    nc = bass.Bass("TRN2", target_bir_lowering=False, debug=False)

    dsr = nc.dram_tensor("dsr", [BPC, DS_LEN], bf16, kind="ExternalInput")
    dsi = nc.dram_tensor("dsi", [BPC, DS_LEN], bf16, kind="ExternalInput")
    scols = nc.dram_tensor("scols", [BPC, 128, 16], f32, kind="ExternalInput")
    tabs = {}
    for r in range(4):
        for form in "AB":
            for h in range(2):
                nm = f"t{form}{r}{h}"
                tabs[(form, r, h)] = nc.dram_tensor(nm, [128, 512], bf16, kind="ExternalInput")
    out = nc.dram_tensor("out", [BPC, K, N], f32, kind="ExternalOutput")

    with tile.TileContext(nc) as tc:
        with (
            tc.tile_pool(name="const", bufs=1) as constp,
            tc.tile_pool(name="win", bufs=2) as winp,
            tc.tile_pool(name="sm", bufs=2) as smp,
            tc.tile_pool(name="u", bufs=2) as up,
            tc.tile_pool(name="pq", bufs=2) as pqp,
            tc.tile_pool(name="bb", bufs=2) as bbp,
            tc.tile_pool(name="sq", bufs=2) as sqp,
            tc.tile_pool(name="chi", bufs=2) as chip,
            tc.tile_pool(name="ps", bufs=2, space="PSUM") as psp,
        ):
            TT = {}
            for i, key in enumerate(tabs):
                TT[key] = constp.tile([128, 512], bf16, tag=f"tab{i}", name=f"tab{i}")

            def load_tables():
                engs = [nc.sync, nc.gpsimd]
                for i, (key, dt_) in enumerate(tabs.items()):
                    engs[i % 2].dma_start(TT[key][:], dt_[:])

            def emit_load(b):
                s = {"b": b}
                Tsr = winp.tile([128, 1536], bf16, tag="tsr", name=f"tsr{b}")
                Tsi = winp.tile([128, 1536], bf16, tag="tsi", name=f"tsi{b}")
                for p0, p1 in ((0, 64), (64, 128)):
                    nc.sync.dma_start(
                        Tsr[p0:p1, :],
                        bass.AP(dsr, b * DS_LEN + 385 + p0, [[1, p1 - p0], [1, 1536]]),
                    )
                    nc.gpsimd.dma_start(
                        Tsi[p0:p1, :],
                        bass.AP(dsi, b * DS_LEN + 385 + p0, [[1, p1 - p0], [1, 1536]]),
                    )
                scol = smp.tile([128, 16], f32, tag="scol", name=f"scol{b}")
                nc.sync.dma_start(scol[:], scols[b])
                s["T"] = (Tsr, Tsi)
                s["scol"] = scol
                s["u"] = {}
                s["B"] = {}
                return s

            def win(T, j, lo, n):
                ap = T[:]
                return bass.AP(ap.tensor, ap.offset + 639 + 128 * j - lo, [ap.ap[0], [-1, n]])

            def emit_ubuild(s, js, lo, hi):
                Tsr, Tsi = s["T"]
                scol = s["scol"]
                n = hi - lo
                for j in js:
                    w_sr = win(Tsr, j, lo, n)
                    w_si = win(Tsi, j, lo, n)
                    sr_c = scol[:, j:j + 1]
                    si_c = scol[:, 8 + j:9 + j]
                    if lo == 0:
                        ut = up.tile([128, 2 * K], bf16, tag=f"u{j}", name=f"u{j}_{s['b']}")
                        s["u"][j] = ut
                    else:
                        ut = s["u"][j]
                    ure = ut[:, lo:hi]
                    uim = ut[:, K + lo:K + hi]
                    a = pqp.tile([128, K], bf16, tag="ta", name=f"ta{s['b']}{j}{lo}")
                    if s["b"] == 0:
                        nc.scalar.mul(a[:, lo:hi], w_si, si_c)
                    else:
                        nc.vector.tensor_scalar_mul(a[:, lo:hi], w_si, si_c)
                    nc.vector.scalar_tensor_tensor(
                        ure, w_sr, sr_c, a[:, lo:hi], op0=ALU.mult, op1=ALU.add
                    )
                    b2 = pqp.tile([128, K], bf16, tag="tb", name=f"tb{s['b']}{j}{lo}")
                    nc.vector.tensor_scalar_mul(b2[:, lo:hi], w_si, sr_c)
                    nc.vector.scalar_tensor_tensor(
                        uim, w_sr, si_c, b2[:, lo:hi], op0=ALU.mult, op1=ALU.subtract
                    )

            def emit_fft4(s, h, lo, hi):
                # B_r[h-chunk] = sum_c (-i)^{cr} u_{j=2c+h}; tiles pack (re|im)
                b = s["b"]
                u0 = s["u"][h]
                u1 = s["u"][2 + h]
                u2 = s["u"][4 + h]
                u3 = s["u"][6 + h]
                t = {}
                for nm in ("P", "Q", "U", "W"):
                    t[nm] = pqp.tile([128, 2 * K], bf16, tag=f"{nm}{h}", name=f"{nm}{h}_{b}")
                Bt = {}
                for r in range(4):
                    Bt[r] = bbp.tile([128, 2 * K], bf16, tag=f"b{r}{h}", name=f"b{r}{h}_{b}")
                s["B"][h] = Bt
                tt = nc.vector.tensor_tensor
                tt(t["P"][:], u0[:], u2[:], op=ALU.add)
                tt(t["Q"][:], u0[:], u2[:], op=ALU.subtract)
                tt(t["U"][:], u1[:], u3[:], op=ALU.add)
                # W = (V.im | -V.re) where V = u1 - u3, so B1 = Q+W, B3 = Q-W
                tt(t["W"][:, 0:K], u1[:, K:2 * K], u3[:, K:2 * K], op=ALU.subtract)
                tt(t["W"][:, K:2 * K], u3[:, 0:K], u1[:, 0:K], op=ALU.subtract)
                tt(Bt[0][:], t["P"][:], t["U"][:], op=ALU.add)
                tt(Bt[2][:], t["P"][:], t["U"][:], op=ALU.subtract)
                tt(Bt[1][:], t["Q"][:], t["W"][:], op=ALU.add)
                tt(Bt[3][:], t["Q"][:], t["W"][:], op=ALU.subtract)

            def emit_kblock(s, kb):
                b = s["b"]
                c0 = 128 * kb
                chi_t = chip.tile([128, N], f32, tag=f"chi{kb % 2}", name=f"chi{b}{kb}")
                for r in range(4):
                    ps = psp.tile([128, 512], f32, tag=f"ps{r}", name=f"ps{b}{kb}{r}")
                    first = True
                    for h in range(2):
                        st = s["B"][h][r][:, c0:c0 + 128]
                        nc.tensor.matmul(ps[:], st, TT[("A", r, h)][:], start=first, stop=False)
                        first = False
                    for h in range(2):
                        st = s["B"][h][r][:, K + c0:K + c0 + 128]
                        nc.tensor.matmul(ps[:], st, TT[("B", r, h)][:], start=False, stop=(h == 1))
                    sq = sqp.tile([128, 512], f32, tag=f"sq{r}", name=f"sq{b}{kb}{r}")
                    nc.scalar.square(sq[:], ps[:])
                    cap = chi_t[:]
                    strided = bass.AP(cap.tensor, cap.offset + r, [cap.ap[0], [4, 256]])
                    eng = nc.vector if (b == 1 and kb == 3) else nc.gpsimd
                    eng.tensor_tensor(strided, sq[:, 0:256], sq[:, 256:512], op=ALU.add)
                return chi_t

            def emit_store(s, kb, chi_t):
                b = s["b"]
                eng = nc.sync if kb % 2 == 0 else nc.scalar
                eng.dma_start(out[b, 128 * kb:128 * kb + 128, :], chi_t[:])

            # ---- schedule ----
            s0 = emit_load(0)
            emit_ubuild(s0, range(8), 0, 512)
            emit_fft4(s0, 0, 0, 512)
            emit_fft4(s0, 1, 0, 512)
            s1 = emit_load(1)
            load_tables()
            c00 = emit_kblock(s0, 0)
            emit_ubuild(s1, range(3), 0, 512)
            emit_store(s0, 0, c00)
            c01 = emit_kblock(s0, 1)
            emit_ubuild(s1, range(3, 6), 0, 512)
            emit_store(s0, 1, c01)
            c02 = emit_kblock(s0, 2)
            emit_ubuild(s1, range(6, 8), 0, 512)
            emit_fft4(s1, 0, 0, 512)
            emit_store(s0, 2, c02)
            c03 = emit_kblock(s0, 3)
            emit_fft4(s1, 1, 0, 512)
            emit_store(s0, 3, c03)
            c10 = emit_kblock(s1, 0)
            emit_store(s1, 0, c10)
            c11 = emit_kblock(s1, 1)
            emit_store(s1, 1, c11)
            c12 = emit_kblock(s1, 2)
            emit_store(s1, 2, c12)
            c13 = emit_kblock(s1, 3)
            emit_store(s1, 3, c13)

    _split_excess_waits(nc)
    return nc


_NC_CACHE = {}


def _get_nc():
    if "nc" not in _NC_CACHE:
        _NC_CACHE["nc"] = build_nc()
    return _NC_CACHE["nc"]


def _get_tables():
    if "tabs" not in _NC_CACHE:
        mpp = np.arange(256, dtype=np.float64)[:, None]
        t = np.arange(256, dtype=np.float64)[None, :]
        t_sh = (t + 128) % 256
        tabs = {}
        for r in range(4):
            ang = 2.0 * np.pi * ((mpp * (r + 4 * t_sh)) % 1024) / 1024
            Mc = np.cos(ang)
            Ms = np.sin(ang)
            for h in range(2):
                sl = slice(128 * h, 128 * h + 128)
                tabs[f"tA{r}{h}"] = np.concatenate(
                    [Mc[sl], -Ms[sl]], axis=1
                ).astype(ml_dtypes.bfloat16)
                tabs[f"tB{r}{h}"] = np.concatenate(
                    [Ms[sl], Mc[sl]], axis=1
                ).astype(ml_dtypes.bfloat16)
        _NC_CACHE["tabs"] = tabs
    return _NC_CACHE["tabs"]


def _host_prep(sr, si):
    """Per-core input prep. sr/si: [BPC, N] float32 (already prescaled)."""
    dsr = np.tile(sr, (1, 3))[:, :DS_LEN].astype(ml_dtypes.bfloat16)
    dsi = np.tile(si, (1, 3))[:, :DS_LEN].astype(ml_dtypes.bfloat16)
    scols = np.concatenate(
        [
            sr.reshape(BPC, 8, 128).transpose(0, 2, 1),
            si.reshape(BPC, 8, 128).transpose(0, 2, 1),
        ],
        axis=2,
    ).astype(np.float32).copy()
    im = {"dsr": dsr, "dsi": dsi, "scols": scols}
    im.update(_get_tables())
    return im


def kernel(s_real: np.ndarray, s_imag: np.ndarray) -> np.ndarray:
    s_real = np.asarray(s_real, dtype=np.float32)
    s_imag = np.asarray(s_imag, dtype=np.float32)
    # exact normalization: max chi = (sum |s|^2)^2 (Cauchy-Schwarz, attained
    # at k=0,f=0), so prescale s by (sum|s|^2)^{-1/2}
    pw = (s_real.astype(np.float64) ** 2 + s_imag.astype(np.float64) ** 2).sum(
        axis=1, keepdims=True
    )
    g = 1.0 / np.sqrt(pw)
    sr_s = (s_real * g).astype(np.float32)
    si_s = (s_imag * g).astype(np.float32)

    nc = _get_nc()
    in_maps = [
        _host_prep(sr_s[c * BPC:(c + 1) * BPC], si_s[c * BPC:(c + 1) * BPC])
        for c in range(NCORES)
    ]
    res = bass_utils.run_bass_kernel_spmd(nc, in_maps, core_ids=list(range(NCORES)))
    chi = np.concatenate([r["out"] for r in res.results], axis=0)  # [B, 512, N]

    full = np.empty((B, N, N), dtype=np.float32)
    full[:, 512:1024, :] = chi
    # mirror: rows r in [1,512): chi[r] = flip_f(chi_direct[512 - r])
    src = chi[:, 511:0:-1, :]                      # k2 = 511..1 -> rows 1..511
    full[:, 1:512, 0] = src[:, :, 0]
    full[:, 1:512, 1:] = src[:, :, :0:-1]
    # row 0 (k=512) on host in float64
    s64 = (sr_s.astype(np.float64) + 1j * si_s.astype(np.float64))
    r512 = s64 * np.conj(np.roll(s64, 512, axis=1))
    x512 = np.fft.fft(r512, axis=1)
    full[:, 0, :] = np.fft.fftshift(
        (x512 * np.conj(x512)).real, axes=-1
    ).astype(np.float32)
    return full


# revision 6
# speedup vs baseline: 1.3442x; 1.0173x over previous
"""Radix-4 DIF ambiguity kernel.

Per batch: u_c = s[m]*conj(s[m-k]) sliding-window products (DVE, bf16),
FFT4 combine over c (DVE, bf16), then 4 branch DFT-256 matmuls with
re/im-concatenated bf16 tables (PE, 512-wide moving), |X|^2 via ACT squares
+ DVE/Pool pair adds. Normalization is exact-by-construction (Cauchy-Schwarz:
max chi = (sum|s|^2)^2) and folded into a host prescale of s. Only k in
[0,512) is computed on device; row k=512 and the mirror half-plane
chi[k,f] = chi[N-k, -f] are assembled during host-side unsharding.
"""

import numpy as np
import ml_dtypes

import bass_rust
import concourse.bass as bass
import concourse.mybir as mybir
import concourse.tile as tile
import concourse.bass_utils as bass_utils

B, N = 16, 1024
NCORES = 8
BPC = B // NCORES
K = 512
DS_LEN = 2176

f32 = mybir.dt.float32
bf16 = mybir.dt.bfloat16
ALU = mybir.AluOpType


def _split_excess_waits(nc):
    for f in nc.m.functions:
        for blk in f.blocks:
            insts = list(blk.instructions)
            new_insts = []
            changed = False
            for inst in insts:
                si = inst.sync_info
                waits = list(si.on_wait) if (si is not None and si.on_wait) else []
                keep_n = 0 if isinstance(inst, mybir.InstDrain) else 1
                if len(waits) > keep_n:
                    changed = True
                    extra = waits[: len(waits) - keep_n]
                    keep = waits[len(waits) - keep_n:]
                    for w in extra:
                        nop = mybir.InstNoOp(
                            name=nc.get_next_instruction_name(), ins=[], outs=[]
                        )
                        nop.engine = inst.engine
                        nop.sync_info = bass_rust.SyncInfo(on_wait=[w], on_update=[])
                        new_insts.append(nop)
                    inst.sync_info = bass_rust.SyncInfo(
                        on_wait=keep,
                        on_update=list(si.on_update) if si.on_update else [],
                    )
                new_insts.append(inst)
            if changed:
                blk.instructions = new_insts
    return nc


def build_nc():
    nc = bass.Bass("TRN2", target_bir_lowering=False, debug=False)

    dsr = nc.dram_tensor("dsr", [BPC, DS_LEN], bf16, kind="ExternalInput")
    dsi = nc.dram_tensor("dsi", [BPC, DS_LEN], bf16, kind="ExternalInput")
    scols = nc.dram_tensor("scols", [BPC, 128, 16], f32, kind="ExternalInput")
    tabs = {}
    for r in range(4):
        for form in "AB":
            for h in range(2):
                nm = f"t{form}{r}{h}"
                tabs[(form, r, h)] = nc.dram_tensor(nm, [128, 512], bf16, kind="ExternalInput")
    out = nc.dram_tensor("out", [BPC, K, N], f32, kind="ExternalOutput")

    with tile.TileContext(nc) as tc:
        with (
            tc.tile_pool(name="const", bufs=1) as constp,
            tc.tile_pool(name="win", bufs=2) as winp,
            tc.tile_pool(name="sm", bufs=2) as smp,
            tc.tile_pool(name="u", bufs=2) as up,
            tc.tile_pool(name="pq", bufs=3) as pqp,
            tc.tile_pool(name="bb", bufs=2) as bbp,
            tc.tile_pool(name="sq", bufs=2) as sqp,
            tc.tile_pool(name="chi", bufs=2) as chip,
            tc.tile_pool(name="ps", bufs=2, space="PSUM") as psp,
        ):
            TT = {}
            for i, key in enumerate(tabs):
                TT[key] = constp.tile([128, 512], bf16, tag=f"tab{i}", name=f"tab{i}")

            def load_tables():
                engs = [nc.sync, nc.gpsimd]
                for i, (key, dt_) in enumerate(tabs.items()):
                    engs[i % 2].dma_start(TT[key][:], dt_[:])

            def emit_load(b):
                s = {"b": b}
                Tsr = winp.tile([128, 1536], bf16, tag="tsr", name=f"tsr{b}")
                Tsi = winp.tile([128, 1536], bf16, tag="tsi", name=f"tsi{b}")
                for p0, p1 in ((0, 64), (64, 128)):
                    nc.sync.dma_start(
                        Tsr[p0:p1, :],
                        bass.AP(dsr, b * DS_LEN + 385 + p0, [[1, p1 - p0], [1, 1536]]),
                    )
                    nc.gpsimd.dma_start(
                        Tsi[p0:p1, :],
                        bass.AP(dsi, b * DS_LEN + 385 + p0, [[1, p1 - p0], [1, 1536]]),
                    )
                scol = smp.tile([128, 16], f32, tag="scol", name=f"scol{b}")
                nc.sync.dma_start(scol[:], scols[b])
                s["T"] = (Tsr, Tsi)
                s["scol"] = scol
                s["u"] = {}
                s["B"] = {}
                return s

            def win(T, j, lo, n):
                ap = T[:]
                return bass.AP(ap.tensor, ap.offset + 639 + 128 * j - lo, [ap.ap[0], [-1, n]])

            def emit_ubuild(s, js, lo, hi):
                Tsr, Tsi = s["T"]
                scol = s["scol"]
                n = hi - lo
                for j in js:
                    w_sr = win(Tsr, j, lo, n)
                    w_si = win(Tsi, j, lo, n)
                    sr_c = scol[:, j:j + 1]
                    si_c = scol[:, 8 + j:9 + j]
                    if lo == 0:
                        ut = up.tile([128, 2 * K], bf16, tag=f"u{j}", name=f"u{j}_{s['b']}")
                        s["u"][j] = ut
                    else:
                        ut = s["u"][j]
                    ure = ut[:, lo:hi]
                    uim = ut[:, K + lo:K + hi]
                    # no-stt form: ACT does the w_si muls, DVE does w_sr muls +
                    # fast-mode adds (stt has no 2x mode; tsm+tt does)
                    a = pqp.tile([128, K], bf16, tag="ta", name=f"ta{s['b']}{j}{lo}")
                    nc.scalar.mul(a[:, lo:hi], w_si, si_c)
                    b2 = pqp.tile([128, K], bf16, tag="tb", name=f"tb{s['b']}{j}{lo}")
                    nc.scalar.mul(b2[:, lo:hi], w_si, sr_c)
                    m1 = pqp.tile([128, K], bf16, tag="tm1", name=f"tm1{s['b']}{j}{lo}")
                    nc.vector.tensor_scalar_mul(m1[:, lo:hi], w_sr, sr_c)
                    nc.vector.tensor_tensor(ure, m1[:, lo:hi], a[:, lo:hi], op=ALU.add)
                    m2 = pqp.tile([128, K], bf16, tag="tm2", name=f"tm2{s['b']}{j}{lo}")
                    nc.vector.tensor_scalar_mul(m2[:, lo:hi], w_sr, si_c)
                    nc.vector.tensor_tensor(uim, m2[:, lo:hi], b2[:, lo:hi], op=ALU.subtract)

            def emit_fft4(s, h, lo, hi):
                # B_r[h-chunk] = sum_c (-i)^{cr} u_{j=2c+h}; tiles pack (re|im)
                b = s["b"]
                u0 = s["u"][h]
                u1 = s["u"][2 + h]
                u2 = s["u"][4 + h]
                u3 = s["u"][6 + h]
                t = {}
                for nm in ("P", "Q", "U", "W"):
                    t[nm] = pqp.tile([128, 2 * K], bf16, tag=f"{nm}{h}", name=f"{nm}{h}_{b}")
                Bt = {}
                for r in range(4):
                    Bt[r] = bbp.tile([128, 2 * K], bf16, tag=f"b{r}{h}", name=f"b{r}{h}_{b}")
                s["B"][h] = Bt
                tt = nc.vector.tensor_tensor
                tt(t["P"][:], u0[:], u2[:], op=ALU.add)
                tt(t["Q"][:], u0[:], u2[:], op=ALU.subtract)
                tt(t["U"][:], u1[:], u3[:], op=ALU.add)
                # W = (V.im | -V.re) where V = u1 - u3, so B1 = Q+W, B3 = Q-W
                tt(t["W"][:, 0:K], u1[:, K:2 * K], u3[:, K:2 * K], op=ALU.subtract)
                tt(t["W"][:, K:2 * K], u3[:, 0:K], u1[:, 0:K], op=ALU.subtract)
                tt(Bt[0][:], t["P"][:], t["U"][:], op=ALU.add)
                tt(Bt[2][:], t["P"][:], t["U"][:], op=ALU.subtract)
                tt(Bt[1][:], t["Q"][:], t["W"][:], op=ALU.add)
                tt(Bt[3][:], t["Q"][:], t["W"][:], op=ALU.subtract)

            def emit_kblock(s, kb):
                b = s["b"]
                c0 = 128 * kb
                chi_t = chip.tile([128, N], f32, tag=f"chi{kb % 2}", name=f"chi{b}{kb}")
                for r in range(4):
                    ps = psp.tile([128, 512], f32, tag=f"ps{r}", name=f"ps{b}{kb}{r}")
                    first = True
                    for h in range(2):
                        st = s["B"][h][r][:, c0:c0 + 128]
                        nc.tensor.matmul(ps[:], st, TT[("A", r, h)][:], start=first, stop=False)
                        first = False
                    for h in range(2):
                        st = s["B"][h][r][:, K + c0:K + c0 + 128]
                        nc.tensor.matmul(ps[:], st, TT[("B", r, h)][:], start=False, stop=(h == 1))
                    sq = sqp.tile([128, 512], f32, tag=f"sq{r}", name=f"sq{b}{kb}{r}")
                    nc.scalar.square(sq[:], ps[:])
                    cap = chi_t[:]
                    strided = bass.AP(cap.tensor, cap.offset + r, [cap.ap[0], [4, 256]])
                    eng = nc.vector if (b == 1 and kb == 3) else nc.gpsimd
                    eng.tensor_tensor(strided, sq[:, 0:256], sq[:, 256:512], op=ALU.add)
                return chi_t

            def emit_store(s, kb, chi_t):
                b = s["b"]
                eng = nc.sync if kb % 2 == 0 else nc.scalar
                eng.dma_start(out[b, 128 * kb:128 * kb + 128, :], chi_t[:])

            # ---- schedule ----
            s0 = emit_load(0)
            emit_ubuild(s0, range(8), 0, 512)
            emit_fft4(s0, 0, 0, 512)
            emit_fft4(s0, 1, 0, 512)
            s1 = emit_load(1)
            load_tables()
            c00 = emit_kblock(s0, 0)
            emit_ubuild(s1, range(3), 0, 512)
            emit_store(s0, 0, c00)
            c01 = emit_kblock(s0, 1)
            emit_ubuild(s1, range(3, 6), 0, 512)
            emit_store(s0, 1, c01)
            c02 = emit_kblock(s0, 2)
            emit_ubuild(s1, range(6, 8), 0, 512)
            emit_fft4(s1, 0, 0, 512)
            emit_store(s0, 2, c02)
            c03 = emit_kblock(s0, 3)
            emit_fft4(s1, 1, 0, 512)
            emit_store(s0, 3, c03)
            c10 = emit_kblock(s1, 0)
            emit_store(s1, 0, c10)
            c11 = emit_kblock(s1, 1)
            emit_store(s1, 1, c11)
            c12 = emit_kblock(s1, 2)
            emit_store(s1, 2, c12)
            c13 = emit_kblock(s1, 3)
            emit_store(s1, 3, c13)

    _split_excess_waits(nc)
    return nc


_NC_CACHE = {}


def _get_nc():
    if "nc" not in _NC_CACHE:
        _NC_CACHE["nc"] = build_nc()
    return _NC_CACHE["nc"]


def _get_tables():
    if "tabs" not in _NC_CACHE:
        mpp = np.arange(256, dtype=np.float64)[:, None]
        t = np.arange(256, dtype=np.float64)[None, :]
        t_sh = (t + 128) % 256
        tabs = {}
        for r in range(4):
            ang = 2.0 * np.pi * ((mpp * (r + 4 * t_sh)) % 1024) / 1024
            Mc = np.cos(ang)
            Ms = np.sin(ang)
            for h in range(2):
                sl = slice(128 * h, 128 * h + 128)
                tabs[f"tA{r}{h}"] = np.concatenate(
                    [Mc[sl], -Ms[sl]], axis=1
                ).astype(ml_dtypes.bfloat16)
                tabs[f"tB{r}{h}"] = np.concatenate(
                    [Ms[sl], Mc[sl]], axis=1
                ).astype(ml_dtypes.bfloat16)
        _NC_CACHE["tabs"] = tabs
    return _NC_CACHE["tabs"]


def _host_prep(sr, si):
    """Per-core input prep. sr/si: [BPC, N] float32 (already prescaled)."""
    dsr = np.tile(sr, (1, 3))[:, :DS_LEN].astype(ml_dtypes.bfloat16)
    dsi = np.tile(si, (1, 3))[:, :DS_LEN].astype(ml_dtypes.bfloat16)
    scols = np.concatenate(
        [
            sr.reshape(BPC, 8, 128).transpose(0, 2, 1),
            si.reshape(BPC, 8, 128).transpose(0, 2, 1),
        ],
        axis=2,
    ).astype(np.float32).copy()
    im = {"dsr": dsr, "dsi": dsi, "scols": scols}
    im.update(_get_tables())
    return im


def kernel(s_real: np.ndarray, s_imag: np.ndarray) -> np.ndarray:
    s_real = np.asarray(s_real, dtype=np.float32)
    s_imag = np.asarray(s_imag, dtype=np.float32)
    # exact normalization: max chi = (sum |s|^2)^2 (Cauchy-Schwarz, attained
    # at k=0,f=0), so prescale s by (sum|s|^2)^{-1/2}
    pw = (s_real.astype(np.float64) ** 2 + s_imag.astype(np.float64) ** 2).sum(
        axis=1, keepdims=True
    )
    g = 1.0 / np.sqrt(pw)
    sr_s = (s_real * g).astype(np.float32)
    si_s = (s_imag * g).astype(np.float32)

    nc = _get_nc()
    in_maps = [
        _host_prep(sr_s[c * BPC:(c + 1) * BPC], si_s[c * BPC:(c + 1) * BPC])
        for c in range(NCORES)
    ]
    res = bass_utils.run_bass_kernel_spmd(nc, in_maps, core_ids=list(range(NCORES)))
    chi = np.concatenate([r["out"] for r in res.results], axis=0)  # [B, 512, N]

    full = np.empty((B, N, N), dtype=np.float32)
    full[:, 512:1024, :] = chi
    # mirror: rows r in [1,512): chi[r] = flip_f(chi_direct[512 - r])
    src = chi[:, 511:0:-1, :]                      # k2 = 511..1 -> rows 1..511
    full[:, 1:512, 0] = src[:, :, 0]
    full[:, 1:512, 1:] = src[:, :, :0:-1]
    # row 0 (k=512) on host in float64
    s64 = (sr_s.astype(np.float64) + 1j * si_s.astype(np.float64))
    r512 = s64 * np.conj(np.roll(s64, 512, axis=1))
    x512 = np.fft.fft(r512, axis=1)
    full[:, 0, :] = np.fft.fftshift(
        (x512 * np.conj(x512)).real, axes=-1
    ).astype(np.float32)
    return full
